# revision 17
# baseline (speedup 1.0000x reference)
"""Distributed causal MultiHeadAttention kernel for 8 Trainium2 NeuronCores.

Problem: B=4, S=2048, D=1024, H=16 heads, dk=dv=64, causal mask, fp32 I/O.

Sharding: data-parallel over batch (4) x tensor-parallel over heads (2 groups
of 8) = 8 cores. Core c handles batch c//2 with heads (c%2)*8 .. (c%2)*8+7.
Each core computes a partial output [S, D] (its head group's contribution
through the corresponding w_o rows); the host sums the pair of partials per
batch (the "all-reduce" of the output projection, done host-side).

Device dataflow (all matmuls bf16 with fp32 PSUM accumulation):
  - Inputs arrive pre-cast to bf16 and pre-packed into the SBUF-native
    [128, chunk, free] partition-major layout host-side.
  - Emission is interleaved at (q-tile, head-pair) granularity so the
    dependency-driven Tile list scheduler overlaps the ACT-bound attention
    inner loop (~157us of ScalarE exp) with the PE-bound projections:
    the kT/qT rows a block needs are emitted immediately before it, so
    ScalarE is saturated from ~7us onward while the PE fills its gaps with
    upcoming projections and the previous q-tile's output projection.
  - Input loads are single whole-slice DMAs split across the two hardware
    DGE queues (SP + Activation) so the k and q streams arrive in parallel.
  - qT = wq.T @ xT per 512-col tile; v = xT.T @ wv with a constant 1.0
    column appended per head ([S, 8, 65]) so the A@V matmul also produces
    the softmax row sums ("ones trick").
  - Scores per head pair: S^T[k, q] = kT.T @ qT; the even/odd head of each
    128-row chunk sits at partitions 0-63 / 64-127, so the two matmuls
    (contract dim 64) row-tile onto disjoint PE quadrants and run
    concurrently, writing the two banks of one [128, 2, 512] PSUM tile.
  - One exp per (pair, q-tile, k-chunk) on ScalarE straight out of PSUM
    (scale=1/8 folded in; no max subtraction - scores are O(1) bounded).
    Causal triangular 128x128 block masked post-exp via gpsimd
    affine_select; fully-masked column ranges are skipped outright.
  - out^T[dv(+1), q] accumulated over k-chunks: lhsT = [V_h | 1], rhs = A^T.
    Row 64 of the PSUM result is the softmax denominator r[q]; each head
    pair's reciprocal runs as a small per-pair chain (DRAM-bounce reshape
    to 32 partitions, bf16 reciprocal, broadcast back over 64 partitions)
    that hides behind the next attention block; the final pair's chain and
    the last q-tile's output stores ride the Activation DGE queue, which is
    idle by then.
"""

import numpy as np
import ml_dtypes

import concourse.bass as bass
import concourse.bacc as bacc
import concourse.mybir as mybir
import concourse.tile as tile
from concourse.bass_utils import run_bass_kernel_spmd

B, S, D = 4, 2048, 1024
H, DK = 16, 64
HL = 8              # heads handled per core
NHL = HL * DK       # 512 rows of head-dim per core
P = 128
NCORES = 8
ST = 512            # q-tile width (matmul free dim / PSUM bank)
NQT = S // ST       # 4
NKC = S // P        # 16 k chunks
MC = NHL // P       # 4 head-pair chunks
DC = D // P         # 8 chunks of D

FP32 = mybir.dt.float32
BF16 = mybir.dt.bfloat16
EXP = mybir.ActivationFunctionType.Exp


def _emit(tc):
    nc = tc.nc

    xqT = nc.dram_tensor("xqT", [P, DC, S], BF16, kind="ExternalInput").ap()
    xkT = nc.dram_tensor("xkT", [P, DC, S], BF16, kind="ExternalInput").ap()
    xvT = nc.dram_tensor("xvT", [P, DC, S], BF16, kind="ExternalInput").ap()
    wq = nc.dram_tensor("wq", [P, DC, NHL], BF16, kind="ExternalInput").ap()
    wk = nc.dram_tensor("wk", [P, DC, NHL], BF16, kind="ExternalInput").ap()
    wv = nc.dram_tensor("wv", [P, DC, NHL], BF16, kind="ExternalInput").ap()
    wo = nc.dram_tensor("wo", [P, MC, D], BF16, kind="ExternalInput").ap()
    out = nc.dram_tensor("out", [S, D], FP32, kind="ExternalOutput").ap()

    with (
        tc.tile_pool(name="sing", bufs=1) as sing,
        tc.tile_pool(name="apool", bufs=6) as apool,
        tc.tile_pool(name="rpool", bufs=4) as rpool,
        tc.tile_pool(name="outp", bufs=3) as outp,
        tc.tile_pool(name="xkp", bufs=2) as xkp,
        tc.tile_pool(name="xqp", bufs=2) as xqp,
        tc.tile_pool(name="xvp", bufs=2) as xvp,
        tc.tile_pool(name="dpool", bufs=4, space="DRAM") as dpool,
        tc.tile_pool(name="psS", bufs=2, space="PSUM") as psS,
        tc.tile_pool(name="psO", bufs=2, space="PSUM") as psO,
        tc.tile_pool(name="psP", bufs=2, space="PSUM") as psP,
    ):
        # ---- persistent SBUF tiles -------------------------------------
        wq_sb = sing.tile([P, DC, NHL], BF16, tag="wq_sb")
        wk_sb = sing.tile([P, DC, NHL], BF16, tag="wk_sb")
        wv_sb = sing.tile([P, DC, NHL], BF16, tag="wv_sb")
        wo_sb = sing.tile([P, MC, D], BF16, tag="wo_sb")
        qT = sing.tile([P, MC, S], BF16, tag="qT")
        kT = sing.tile([P, MC, S], BF16, tag="kT")
        v65 = sing.tile([P, NKC, HL, DK + 1], BF16, tag="v65")
        oT = sing.tile([P, MC, S], BF16, tag="oT")

        nc.gpsimd.memset(v65[:, :, :, DK : DK + 1], 1.0)

        # ---- input loads: one whole-slice DMA each, on two HW queues ---
        def load_x_slice(pool, dram_ap, st, tag, eng=None):
            t = pool.tile([P, DC, ST], BF16, tag=tag, name=tag)
            (eng or nc.sync).dma_start(t, dram_ap[:, :, st * ST : (st + 1) * ST])
            return t

        nc.scalar.dma_start(wk_sb, wk)
        xk_t = load_x_slice(xkp, xkT, 0, "xk")
        nc.scalar.dma_start(wq_sb, wq)
        xq_t = load_x_slice(xqp, xqT, 0, "xq")
        nc.scalar.dma_start(wv_sb, wv)
        xv_t = load_x_slice(xvp, xvT, 0, "xv")
        nc.scalar.dma_start(wo_sb, wo)

        # ---- projection units ------------------------------------------
        def proj_qk(w_sb, x_t, dst, st, mc):
            ps = psP.tile([P, ST], FP32, tag="psP")
            for dc in range(DC):
                nc.tensor.matmul(
                    ps,
                    lhsT=w_sb[:, dc, mc * P : (mc + 1) * P],
                    rhs=x_t[:, dc, :],
                    start=(dc == 0),
                    stop=(dc == DC - 1),
                )
            nc.vector.tensor_copy(dst[:, mc, st * ST : (st + 1) * ST], ps)

        def proj_v(x_t, sc):
            ps = psP.tile([P, ST], FP32, tag="psP")
            for dc in range(DC):
                nc.tensor.matmul(
                    ps,
                    lhsT=x_t[:, dc, (sc % 4) * P : (sc % 4 + 1) * P],
                    rhs=wv_sb[:, dc, :],
                    start=(dc == 0),
                    stop=(dc == DC - 1),
                )
            nc.vector.tensor_copy(
                v65[:, sc, :, 0:DK], ps.rearrange("p (h d) -> p h d", h=HL)
            )

        # ---- attention block (qt, pc) ----------------------------------
        # Masked-column skip: for k-chunk kc in q-tile qt, columns below
        # lo = (kc - 4*qt)*128 are entirely masked; scores/exp/A@V all skip
        # them.
        def clo(kc, qt):
            j = kc - qt * (ST // P)
            return j * P if j > 0 else 0

        def emit_av(a_t, kc, oT_ps, pc, qt, nkc):
            lo = clo(kc, qt)
            for hh in range(2):
                nc.tensor.matmul(
                    oT_ps[hh][:, lo:ST],
                    lhsT=v65[:, kc, 2 * pc + hh, :],
                    rhs=a_t[:, hh, lo:ST],
                    start=(kc == 0),
                    stop=(kc == nkc - 1),
                )

        def attn_block(qt, pc):
            # (0,3) is the deferred final block; its chain rides the
            # Activation DGE queue, idle once the exp stream has drained
            tail = qt == 0 and pc == MC - 1
            dma = nc.scalar.dma_start if tail else nc.sync.dma_start
            nkc = (qt + 1) * (ST // P)
            oT_ps = [
                psO.tile([DK + 1, ST], FP32, tag="psO", name=f"psO_{hh}")
                for hh in range(2)
            ]
            prev = []
            for kc in range(nkc):
                lo = clo(kc, qt)
                sps = psS.tile([P, 2, ST], FP32, tag="psS", name="sps")
                for hh in range(2):
                    pp = hh * 64
                    nc.tensor.matmul(
                        sps[:, hh, lo:ST],
                        lhsT=kT[pp : pp + 64, pc, kc * P : (kc + 1) * P],
                        rhs=qT[pp : pp + 64, pc, qt * ST + lo : (qt + 1) * ST],
                        start=True,
                        stop=True,
                    )
                a_t = apool.tile([P, 2, ST], BF16, tag="a", name="a_t")
                nc.scalar.activation(
                    a_t[:, :, lo:ST], sps[:, :, lo:ST], EXP,
                    bias=0.0, scale=0.125,
                )
                if kc >= qt * (ST // P):
                    # triangular block: keep where q_local >= k_local
                    nc.gpsimd.affine_select(
                        out=a_t[:, :, lo : lo + P],
                        in_=a_t[:, :, lo : lo + P],
                        pattern=[[0, 2], [1, P]],
                        channel_multiplier=-1,
                        base=0,
                        compare_op=mybir.AluOpType.is_ge,
                        fill=0.0,
                    )
                prev.append((a_t, kc))
                if len(prev) > 1:
                    a_p, kc_p = prev.pop(0)
                    emit_av(a_p, kc_p, oT_ps, pc, qt, nkc)
            for a_p, kc_p in prev:
                emit_av(a_p, kc_p, oT_ps, pc, qt, nkc)

            # release the accumulators fast: copy raw (unnormalized) oT out
            # and stash the softmax sums; the reciprocal mini-chain hides
            # behind the next attention block.
            rdq = dpool.tile([2, ST], FP32, tag="rdq", name="rdq")
            for hh in range(2):
                ps = oT_ps[hh]
                rsb = rpool.tile([1, ST], FP32, tag="rsb")
                nc.vector.tensor_copy(rsb, ps[DK : DK + 1, :])
                dma(rdq[hh : hh + 1, :], rsb)
                nc.vector.tensor_copy(
                    oT[hh * 64 : (hh + 1) * 64, pc, qt * ST : (qt + 1) * ST],
                    ps[0:DK, :],
                )
            r32 = rpool.tile([2 * 16, ST // 16], FP32, tag="r32")
            dma(r32, rdq.rearrange("a (p f) -> (a p) f", p=16))
            ri32 = rpool.tile([2 * 16, ST // 16], FP32, tag="ri32")
            nc.vector.reciprocal(ri32, r32)
            rdi = dpool.tile([2, ST], FP32, tag="rdi", name="rdi")
            dma(rdi.rearrange("a (p f) -> (a p) f", p=16), ri32)
            rrep = rpool.tile([P, ST], FP32, tag="rrep")
            for hh in range(2):
                dma(
                    rrep[hh * 64 : (hh + 1) * 64, :],
                    rdi[hh : hh + 1, :].to_broadcast((64, ST)),
                )
            sl = oT[:, pc, qt * ST : (qt + 1) * ST]
            nc.vector.tensor_mul(sl, sl, rrep)

        # ---- output projection for one q-tile's rows ---------------------
        def out_qtile(qt):
            tail = qt == 0  # OUT(0) runs last, after deferred (0,3)
            dma = nc.scalar.dma_start if tail else nc.sync.dma_start
            for sc in range(qt * (ST // P), (qt + 1) * (ST // P)):
                for nt in range(D // ST):
                    ps = psP.tile([P, ST], FP32, tag="psP")
                    for c in range(MC):
                        nc.tensor.matmul(
                            ps,
                            lhsT=oT[:, c, sc * P : (sc + 1) * P],
                            rhs=wo_sb[:, c, nt * ST : (nt + 1) * ST],
                            start=(c == 0),
                            stop=(c == MC - 1),
                        )
                    ob = outp.tile([P, ST], FP32, tag="ob")
                    nc.vector.tensor_copy(ob, ps)
                    dma(out[sc * P : (sc + 1) * P, nt * ST : (nt + 1) * ST], ob)

        # ---- interleaved emission: (q-tile, head-pair) granularity ------
        # Block (0,3) is deferred to the very end so the final exp stream is
        # a small 4-chunk block: the post-exp tail becomes its short A@V +
        # reciprocal chain + OUT(0), with OUT(3) overlapping the handoff,
        # instead of the big (3,3) block's chain fully exposed.
        for rnd in range(NQT):
            for pc in range(MC):
                proj_qk(wk_sb, xk_t, kT, rnd, pc)
                proj_qk(wq_sb, xq_t, qT, rnd, pc)
                if pc == 0:
                    for i in range(4):
                        proj_v(xv_t, rnd * 4 + i)
                elif pc == 3 and rnd > 1:
                    out_qtile(rnd - 1)
                if (rnd, pc) != (0, 3):
                    attn_block(rnd, pc)
            if rnd + 1 < NQT:
                xk_t = load_x_slice(xkp, xkT, rnd + 1, "xk")
                xq_t = load_x_slice(xqp, xqT, rnd + 1, "xq")
                xv_t = load_x_slice(xvp, xvT, rnd + 1, "xv")
        out_qtile(NQT - 1)
        attn_block(0, 3)
        out_qtile(0)


_CACHE = {}


def build_nc():
    if "nc" not in _CACHE:
        # Bacc (not plain Bass): its finalize runs the pass pipeline that
        # splits multi-semaphore waits into event-semaphore/ldweights slots,
        # which walrus requires (max 1 wait per instruction on TRN2).
        nc = bacc.Bacc()
        with tile.TileContext(nc) as tc:
            _emit(tc)
        nc.finalize()
        _CACHE["nc"] = nc
    return _CACHE["nc"]


def make_in_maps(query, key, value, w_q, w_k, w_v, w_o):
    bf = ml_dtypes.bfloat16

    def packT(x):  # [S, D] fp32 -> xT packed [128, DC, S] bf16
        xb = np.asarray(x, np.float32).astype(bf)
        return np.ascontiguousarray(xb.T.reshape(DC, P, S).transpose(1, 0, 2))

    def packW(w):  # [D, NHL] -> [128, DC, NHL]
        wb = np.asarray(w, np.float32).astype(bf)
        return np.ascontiguousarray(wb.reshape(DC, P, NHL).transpose(1, 0, 2))

    def packWo(w):  # [NHL, D] -> [128, MC, D]
        wb = np.asarray(w, np.float32).astype(bf)
        return np.ascontiguousarray(wb.reshape(MC, P, D).transpose(1, 0, 2))

    query = np.asarray(query, np.float32)
    key = np.asarray(key, np.float32)
    value = np.asarray(value, np.float32)
    in_maps = []
    for c in range(NCORES):
        b, hg = divmod(c, 2)
        cols = slice(hg * NHL, (hg + 1) * NHL)
        in_maps.append(
            {
                "xqT": packT(query[b]),
                "xkT": packT(key[b]),
                "xvT": packT(value[b]),
                "wq": packW(np.asarray(w_q)[:, cols]),
                "wk": packW(np.asarray(w_k)[:, cols]),
                "wv": packW(np.asarray(w_v)[:, cols]),
                "wo": packWo(np.asarray(w_o)[cols, :]),
            }
        )
    return in_maps


def kernel(query, key, value, mask, w_q, w_k, w_v, w_o, **run_kwargs):
    nc = build_nc()
    in_maps = make_in_maps(query, key, value, w_q, w_k, w_v, w_o)
    res = run_bass_kernel_spmd(nc, in_maps, list(range(NCORES)), **run_kwargs)
    out = np.empty((B, S, D), np.float32)
    for b in range(B):
        out[b] = res.results[2 * b]["out"] + res.results[2 * b + 1]["out"]
    return out


# revision 20
# speedup vs baseline: 1.1701x; 1.1701x over previous
"""Distributed causal MultiHeadAttention kernel for 8 Trainium2 NeuronCores.

Problem: B=4, S=2048, D=1024, H=16 heads, dk=dv=64, causal mask, fp32 I/O.

Sharding: data-parallel over batch (4) x tensor-parallel over heads (2 groups
of 8) = 8 cores. Core c handles batch c//2 with heads (c%2)*8 .. (c%2)*8+7.
Each core computes a partial output [S, D] (its head group's contribution
through the corresponding w_o rows); the host sums the pair of partials per
batch (the "all-reduce" of the output projection, done host-side).

Device dataflow (all matmuls bf16 with fp32 PSUM accumulation):
  - Inputs arrive pre-cast to bf16 and pre-packed into the SBUF-native
    [128, chunk, free] partition-major layout host-side.
  - Emission is interleaved at (q-tile, head-pair) granularity so the
    dependency-driven Tile list scheduler overlaps the ACT-bound attention
    inner loop (~157us of ScalarE exp) with the PE-bound projections:
    the kT/qT rows a block needs are emitted immediately before it, and
    the PE fills its gaps with upcoming projections and the previous
    q-tile's output projection while ScalarE streams exps.
  - Input loads are single whole-slice DMAs split across the two hardware
    DGE queues (SP carries x, Activation carries w) so both streams ride
    HBM in parallel; few descriptors keeps both queues responsive
    mid-kernel (finer-chunked variants measured slower end-to-end).
  - qT = wq.T @ xT per 512-col tile; v = xT.T @ wv with a constant 1.0
    column appended per head ([S, 8, 65]) so the A@V matmul also produces
    the softmax row sums ("ones trick").
  - Scores per head pair: S^T[k, q] = kT.T @ qT; the even/odd head of each
    128-row chunk sits at partitions 0-63 / 64-127, so the two matmuls
    (contract dim 64) row-tile onto disjoint PE quadrants and run
    concurrently, writing the two banks of one [128, 2, 512] PSUM tile.
  - One exp per (pair, q-tile, k-chunk) on ScalarE straight out of PSUM
    (scale=1/8 folded in; no max subtraction - scores are O(1) bounded).
    Causal triangular 128x128 block masked post-exp via gpsimd
    affine_select; fully-masked column ranges are skipped outright.
  - out^T[dv(+1), q] accumulated over k-chunks: lhsT = [V_h | 1], rhs = A^T.
    Row 64 of the PSUM result is the softmax denominator r[q]; each head
    pair's reciprocal runs as a small per-pair chain (DRAM-bounce reshape
    to 32 partitions, fp32 reciprocal, broadcast back over 64 partitions)
    that hides behind the next attention block; the final pair's chain and
    the last q-tile's output stores ride the Activation DGE queue, which is
    idle by then.
"""

import numpy as np
import ml_dtypes

import concourse.bass as bass
import concourse.bacc as bacc
import concourse.mybir as mybir
import concourse.tile as tile
from concourse.bass_utils import run_bass_kernel_spmd

B, S, D = 4, 2048, 1024
H, DK = 16, 64
HL = 8              # heads handled per core
NHL = HL * DK       # 512 rows of head-dim per core
P = 128
NCORES = 8
ST = 512            # q-tile width (matmul free dim / PSUM bank)
NQT = S // ST       # 4
NKC = S // P        # 16 k chunks
MC = NHL // P       # 4 head-pair chunks
DC = D // P         # 8 chunks of D

FP32 = mybir.dt.float32
BF16 = mybir.dt.bfloat16
EXP = mybir.ActivationFunctionType.Exp


def _emit(tc):
    nc = tc.nc

    xqT = nc.dram_tensor("xqT", [P, DC, S], BF16, kind="ExternalInput").ap()
    xkT = nc.dram_tensor("xkT", [P, DC, S], BF16, kind="ExternalInput").ap()
    xvT = nc.dram_tensor("xvT", [P, DC, S], BF16, kind="ExternalInput").ap()
    wq = nc.dram_tensor("wq", [P, DC, NHL], BF16, kind="ExternalInput").ap()
    wk = nc.dram_tensor("wk", [P, DC, NHL], BF16, kind="ExternalInput").ap()
    wv = nc.dram_tensor("wv", [P, DC, NHL], BF16, kind="ExternalInput").ap()
    wo = nc.dram_tensor("wo", [P, MC, D], BF16, kind="ExternalInput").ap()
    out = nc.dram_tensor("out", [S, D], FP32, kind="ExternalOutput").ap()

    with (
        tc.tile_pool(name="sing", bufs=1) as sing,
        tc.tile_pool(name="apool", bufs=6) as apool,
        tc.tile_pool(name="rpool", bufs=4) as rpool,
        tc.tile_pool(name="outp", bufs=3) as outp,
        tc.tile_pool(name="xkp", bufs=2) as xkp,
        tc.tile_pool(name="xqp", bufs=2) as xqp,
        tc.tile_pool(name="xvp", bufs=2) as xvp,
        tc.tile_pool(name="dpool", bufs=4, space="DRAM") as dpool,
        tc.tile_pool(name="psS", bufs=2, space="PSUM") as psS,
        tc.tile_pool(name="psO", bufs=2, space="PSUM") as psO,
        tc.tile_pool(name="psP", bufs=2, space="PSUM") as psP,
    ):
        # ---- persistent SBUF tiles -------------------------------------
        wq_sb = sing.tile([P, DC, NHL], BF16, tag="wq_sb")
        wk_sb = sing.tile([P, DC, NHL], BF16, tag="wk_sb")
        wv_sb = sing.tile([P, DC, NHL], BF16, tag="wv_sb")
        wo_sb = sing.tile([P, MC, D], BF16, tag="wo_sb")
        qT = sing.tile([P, MC, S], BF16, tag="qT")
        kT = sing.tile([P, MC, S], BF16, tag="kT")
        v65 = sing.tile([P, NKC, HL, DK + 1], BF16, tag="v65")
        oT = sing.tile([P, MC, S], BF16, tag="oT")

        nc.gpsimd.memset(v65[:, :, :, DK : DK + 1], 1.0)

        # ---- input loads: one whole-slice DMA each, on two HW queues ---
        def load_x_slice(pool, dram_ap, st, tag, eng=None):
            t = pool.tile([P, DC, ST], BF16, tag=tag, name=tag)
            (eng or nc.sync).dma_start(t, dram_ap[:, :, st * ST : (st + 1) * ST])
            return t

        nc.scalar.dma_start(wk_sb, wk)
        xk_t = load_x_slice(xkp, xkT, 0, "xk")
        nc.scalar.dma_start(wq_sb, wq)
        xq_t = load_x_slice(xqp, xqT, 0, "xq")
        nc.scalar.dma_start(wv_sb, wv)
        xv_t = load_x_slice(xvp, xvT, 0, "xv")
        nc.scalar.dma_start(wo_sb, wo)

        # ---- projection units ------------------------------------------
        def proj_qk(w_sb, x_t, dst, st, mc):
            ps = psP.tile([P, ST], FP32, tag="psP")
            for dc in range(DC):
                nc.tensor.matmul(
                    ps,
                    lhsT=w_sb[:, dc, mc * P : (mc + 1) * P],
                    rhs=x_t[:, dc, :],
                    start=(dc == 0),
                    stop=(dc == DC - 1),
                )
            nc.vector.tensor_copy(dst[:, mc, st * ST : (st + 1) * ST], ps)

        def proj_v(x_t, sc):
            ps = psP.tile([P, ST], FP32, tag="psP")
            for dc in range(DC):
                nc.tensor.matmul(
                    ps,
                    lhsT=x_t[:, dc, (sc % 4) * P : (sc % 4 + 1) * P],
                    rhs=wv_sb[:, dc, :],
                    start=(dc == 0),
                    stop=(dc == DC - 1),
                )
            nc.vector.tensor_copy(
                v65[:, sc, :, 0:DK], ps.rearrange("p (h d) -> p h d", h=HL)
            )

        # ---- attention block (qt, pc) ----------------------------------
        # Masked-column skip: for k-chunk kc in q-tile qt, columns below
        # lo = (kc - 4*qt)*128 are entirely masked; scores/exp/A@V all skip
        # them.
        def clo(kc, qt):
            j = kc - qt * (ST // P)
            return j * P if j > 0 else 0

        def emit_av(a_t, kc, oT_ps, pc, qt, nkc):
            lo = clo(kc, qt)
            for hh in range(2):
                nc.tensor.matmul(
                    oT_ps[hh][:, lo:ST],
                    lhsT=v65[:, kc, 2 * pc + hh, :],
                    rhs=a_t[:, hh, lo:ST],
                    start=(kc == 0),
                    stop=(kc == nkc - 1),
                )

        def attn_block(qt, pc):
            tail = qt == NQT - 1 and pc == MC - 1
            dma = nc.scalar.dma_start if tail else nc.sync.dma_start
            nkc = (qt + 1) * (ST // P)
            oT_ps = [
                psO.tile([DK + 1, ST], FP32, tag="psO", name=f"psO_{hh}")
                for hh in range(2)
            ]
            prev = []
            for kc in range(nkc):
                lo = clo(kc, qt)
                sps = psS.tile([P, 2, ST], FP32, tag="psS", name="sps")
                for hh in range(2):
                    pp = hh * 64
                    nc.tensor.matmul(
                        sps[:, hh, lo:ST],
                        lhsT=kT[pp : pp + 64, pc, kc * P : (kc + 1) * P],
                        rhs=qT[pp : pp + 64, pc, qt * ST + lo : (qt + 1) * ST],
                        start=True,
                        stop=True,
                    )
                a_t = apool.tile([P, 2, ST], BF16, tag="a", name="a_t")
                nc.scalar.activation(
                    a_t[:, :, lo:ST], sps[:, :, lo:ST], EXP,
                    bias=0.0, scale=0.125,
                )
                if kc >= qt * (ST // P):
                    # triangular block: keep where q_local >= k_local
                    nc.gpsimd.affine_select(
                        out=a_t[:, :, lo : lo + P],
                        in_=a_t[:, :, lo : lo + P],
                        pattern=[[0, 2], [1, P]],
                        channel_multiplier=-1,
                        base=0,
                        compare_op=mybir.AluOpType.is_ge,
                        fill=0.0,
                    )
                prev.append((a_t, kc))
                if len(prev) > 1:
                    a_p, kc_p = prev.pop(0)
                    emit_av(a_p, kc_p, oT_ps, pc, qt, nkc)
            for a_p, kc_p in prev:
                emit_av(a_p, kc_p, oT_ps, pc, qt, nkc)

            # release the accumulators fast: copy raw (unnormalized) oT out
            # and stash the softmax sums; the reciprocal mini-chain hides
            # behind the next attention block.
            rdq = dpool.tile([2, ST], FP32, tag="rdq", name="rdq")
            for hh in range(2):
                ps = oT_ps[hh]
                rsb = rpool.tile([1, ST], FP32, tag="rsb")
                nc.vector.tensor_copy(rsb, ps[DK : DK + 1, :])
                dma(rdq[hh : hh + 1, :], rsb)
                nc.vector.tensor_copy(
                    oT[hh * 64 : (hh + 1) * 64, pc, qt * ST : (qt + 1) * ST],
                    ps[0:DK, :],
                )
            r32 = rpool.tile([2 * 16, ST // 16], FP32, tag="r32")
            dma(r32, rdq.rearrange("a (p f) -> (a p) f", p=16))
            ri32 = rpool.tile([2 * 16, ST // 16], FP32, tag="ri32")
            nc.vector.reciprocal(ri32, r32)
            rdi = dpool.tile([2, ST], FP32, tag="rdi", name="rdi")
            dma(rdi.rearrange("a (p f) -> (a p) f", p=16), ri32)
            rrep = rpool.tile([P, ST], FP32, tag="rrep")
            for hh in range(2):
                dma(
                    rrep[hh * 64 : (hh + 1) * 64, :],
                    rdi[hh : hh + 1, :].to_broadcast((64, ST)),
                )
            sl = oT[:, pc, qt * ST : (qt + 1) * ST]
            nc.vector.tensor_mul(sl, sl, rrep)

        # ---- output projection for one q-tile's rows ---------------------
        def out_qtile(qt):
            tail = qt == NQT - 1
            dma = nc.scalar.dma_start if tail else nc.sync.dma_start
            for sc in range(qt * (ST // P), (qt + 1) * (ST // P)):
                for nt in range(D // ST):
                    ps = psP.tile([P, ST], FP32, tag="psP")
                    for c in range(MC):
                        nc.tensor.matmul(
                            ps,
                            lhsT=oT[:, c, sc * P : (sc + 1) * P],
                            rhs=wo_sb[:, c, nt * ST : (nt + 1) * ST],
                            start=(c == 0),
                            stop=(c == MC - 1),
                        )
                    ob = outp.tile([P, ST], FP32, tag="ob")
                    nc.vector.tensor_copy(ob, ps)
                    dma(out[sc * P : (sc + 1) * P, nt * ST : (nt + 1) * ST], ob)

        # ---- interleaved emission: (q-tile, head-pair) granularity ------
        for rnd in range(NQT):
            for pc in range(MC):
                proj_qk(wk_sb, xk_t, kT, rnd, pc)
                proj_qk(wq_sb, xq_t, qT, rnd, pc)
                if pc == 0:
                    for i in range(4):
                        proj_v(xv_t, rnd * 4 + i)
                elif pc == 3 and rnd > 0:
                    out_qtile(rnd - 1)
                attn_block(rnd, pc)
            if rnd + 1 < NQT:
                xk_t = load_x_slice(xkp, xkT, rnd + 1, "xk")
                xq_t = load_x_slice(xqp, xqT, rnd + 1, "xq")
                xv_t = load_x_slice(xvp, xvT, rnd + 1, "xv")
        out_qtile(NQT - 1)


_CACHE = {}


def build_nc():
    if "nc" not in _CACHE:
        # Bacc (not plain Bass): its finalize runs the pass pipeline that
        # splits multi-semaphore waits into event-semaphore/ldweights slots,
        # which walrus requires (max 1 wait per instruction on TRN2).
        nc = bacc.Bacc()
        with tile.TileContext(nc) as tc:
            _emit(tc)
        nc.finalize()
        _CACHE["nc"] = nc
    return _CACHE["nc"]


def make_in_maps(query, key, value, w_q, w_k, w_v, w_o):
    bf = ml_dtypes.bfloat16

    def packT(x):  # [S, D] fp32 -> xT packed [128, DC, S] bf16
        xb = np.asarray(x, np.float32).astype(bf)
        return np.ascontiguousarray(xb.T.reshape(DC, P, S).transpose(1, 0, 2))

    def packW(w):  # [D, NHL] -> [128, DC, NHL]
        wb = np.asarray(w, np.float32).astype(bf)
        return np.ascontiguousarray(wb.reshape(DC, P, NHL).transpose(1, 0, 2))

    def packWo(w):  # [NHL, D] -> [128, MC, D]
        wb = np.asarray(w, np.float32).astype(bf)
        return np.ascontiguousarray(wb.reshape(MC, P, D).transpose(1, 0, 2))

    query = np.asarray(query, np.float32)
    key = np.asarray(key, np.float32)
    value = np.asarray(value, np.float32)
    in_maps = []
    for c in range(NCORES):
        b, hg = divmod(c, 2)
        cols = slice(hg * NHL, (hg + 1) * NHL)
        in_maps.append(
            {
                "xqT": packT(query[b]),
                "xkT": packT(key[b]),
                "xvT": packT(value[b]),
                "wq": packW(np.asarray(w_q)[:, cols]),
                "wk": packW(np.asarray(w_k)[:, cols]),
                "wv": packW(np.asarray(w_v)[:, cols]),
                "wo": packWo(np.asarray(w_o)[cols, :]),
            }
        )
    return in_maps


def kernel(query, key, value, mask, w_q, w_k, w_v, w_o, **run_kwargs):
    nc = build_nc()
    in_maps = make_in_maps(query, key, value, w_q, w_k, w_v, w_o)
    res = run_bass_kernel_spmd(nc, in_maps, list(range(NCORES)), **run_kwargs)
    out = np.empty((B, S, D), np.float32)
    for b in range(B):
        out[b] = res.results[2 * b]["out"] + res.results[2 * b + 1]["out"]
    return out
